# revision 61
# baseline (speedup 1.0000x reference)
"""Trainium2 Bass kernel for nn_LossFunction_2740189135094 (AAM-softmax +
score-normalized angle-proto speaker loss).

Contract: kernel(**inputs) takes FULL unsharded inputs (as produced by the
reference setup_inputs) and returns the full output: a (2,) float32 array
[nlossS + nlossP, prec1].

Strategy (8 NeuronCores, class-sharded; tiny partial outputs merged on host):
  Each core owns 752 of the (padded-to-6016) 5994 classes and 256 of the
  2048 anchors. Per 128-row tile the PE computes, in one fused [128, 1008]
  PSUM tile, cosine vs the class shard (fp8 DoubleRow) and - for positive
  rows - the angle-proto similarity D vs the anchor shard. The row-wise
  sum(exp(30*cos)) is computed by BOTH the ACT engine (true exp + fused
  accumulate) and the DVE (Schraudolph bit-trick exp: affine f32->uint16,
  bitcast to bf16, 4x-mode accumulate), split per a static assignment so the
  two engines finish together. exp(alpha*D) row sums come from the same
  machinery; anchor-column sums come from a ones-vector matmul on the PE
  accumulated across row tiles in PSUM.

  There is no on-device row max: prec1 = mean(phi > max_{j!=label} cos_j)
  is decided on the host from the row-wise sum(exp): max >= log(sum/5993)/30.
  For this loss phi sits far below that bound; rows too close to the bound
  (never, in practice) fall back to an exact host check.

  Host does: l2-normalization, fp8-e4m3 quantization, layout packing (so
  every DMA is contiguous per partition - the input load is descriptor-bound
  otherwise), the label-gathered target cosines from the same fp8 operands,
  and the final logs/means in float64.

The top-k cohort statistics in the reference are multiplied by w2/b2; for
the actual inputs w2 == b2 == 0, so csm is an affine function of out_dot and
p2's matrix is exactly p1's transpose. If w2/b2 were nonzero we fall back to
an exact numpy implementation.
"""

import math
import sys

import numpy as np

for _p in ("/opt/trn_rl_repo", "/opt/pypackages"):
    if _p not in sys.path:
        sys.path.insert(0, _p)

import ml_dtypes  # noqa: E402

NOUT = 512
NCLS = 5994
B = 2048
R = 4096  # 2 * B rows
NCORES = 8
CSH = 752  # padded class shard: 8 * 752 = 6016 >= 5994
NPAD = NCORES * CSH - NCLS  # 22 zero-padded classes on the last core
ASH = B // NCORES  # 256 anchors per core
MARGIN = 0.2
SCALE = 30.0
PSW = CSH + ASH  # fused psum tile width: classes + anchor shard

_COS_M = math.cos(MARGIN)
_SIN_M = math.sin(MARGIN)
_TH = math.cos(math.pi - MARGIN)
_MM = math.sin(math.pi - MARGIN) * MARGIN

LOG2E = 1.4426950408889634
# Schraudolph bf16 exp: exp(s*c) ~= bitcast_bf16(uint16(c*(128*s*log2e) + BC)).
# BC is bias-free for sums: 128*(127 - log2(E_f[(1+f)/2^f])).
SCH_B = 128.0 * 127.0 - 128.0 * math.log2(1.0407419227)

# Engine assignment: 'A' = ACT (true exp, fused accum), 'D' = DVE
# (Schraudolph affine+convert + bf16 accumulate; the Pool engine supports
# neither PSUM access nor TensorScalar, so only these two engines can
# consume matmul results). Balanced from measured slice times: ACT class
# tile ~1069ns (exp 861 + accum read 208), DVE class pair ~1792ns (the
# accumulate lowers to TENSOR_SCALAR_CACHE_REDUCE, which runs 1 elem/cycle
# - no 2x/4x modes), ACT D tile ~630ns, DVE D pair ~670ns.
N_ACT_CLS = 22
ASSIGN_CLS = [
    "A" if (i * N_ACT_CLS) // 32 != ((i + 1) * N_ACT_CLS) // 32 else "D"
    for i in range(32)
]
# rt0 on ACT (it can start right after the early table load); rt30 on DVE
# (the formula puts three ACT tiles in a row at the tail, serializing it).
ASSIGN_CLS[0], ASSIGN_CLS[30] = "A", "D"
# The D-tile 'A' slots sit on ALL DVE-class rows so every PSUM tile's two
# consumers run on different engines concurrently (a tile whose class pair
# AND D pair both land on DVE holds the 3-deep PSUM pool ~2.4us and stalls
# the PE).
_dve_cls_rts = [i for i in range(16) if ASSIGN_CLS[i] == "D"]
ASSIGN_D = ["A" if i in set(_dve_cls_rts) else "D" for i in range(16)]
# A matmul instruction costs ~N_out cycles regardless of contraction depth
# (DoubleRow packs k=256 per instruction; it does NOT halve the per-
# instruction stream time), so the PE floor here is ~25us and the PE is the
# natural pacer. One small heater matmul per row tile keeps the HAM clock
# governor at 2.4 GHz (without them the clock was observed stuck at
# 0.65-1.2 GHz for the whole run, doubling every matmul).
N_HEAT = 1
HEAT_N = 128
N_WARM = 4

_cache: dict = {}

# Results of the last device run (for the test harness to inspect timing).
last_results = None


def _hsig(v):
    return np.clip((v + 3.0) / 6.0, 0.0, 1.0)


def _sch_exp_np(c, s):
    """Replicate the device's Schraudolph exp in numpy (float64 out)."""
    a = np.float32(128.0 * s * LOG2E)
    t = np.asarray(c, np.float32) * a + np.float32(SCH_B)
    i = np.round(t).astype(np.uint16)
    return i.view(ml_dtypes.bfloat16).astype(np.float64)


def _build_program(alpha):
    import concourse.mybir as mybir
    import concourse.tile as tile
    from concourse import bacc
    from contextlib import ExitStack

    bf16 = mybir.dt.bfloat16
    u16 = mybir.dt.uint16
    f8 = mybir.dt.float8e4
    f32 = mybir.dt.float32
    DR = mybir.MatmulPerfMode.DoubleRow
    EXP = mybir.ActivationFunctionType.Exp
    MULT = mybir.AluOpType.mult
    ADD = mybir.AluOpType.add
    MAX = mybir.AluOpType.max

    sch_a_cls = float(np.float32(128.0 * SCALE * LOG2E))
    sch_a_d = float(np.float32(128.0 * alpha * LOG2E))
    sch_b = float(np.float32(SCH_B))

    nc = bacc.Bacc(
        "TRN2", target_bir_lowering=False, debug=False, num_devices=NCORES
    )
    # Inputs are pre-packed on host so each DMA is one contiguous run per
    # partition (the load is descriptor-count-bound otherwise).
    # Layout [p][q][c][r][m]: element = operand[k, col] with k = c*256+r*128+p
    # (contraction index, DoubleRow-packed) and col = q*512+m.
    # wtx fuses the class tail (240 cols) with the anchor shard (256 cols) so
    # PSUM bank 1 holds exactly one accumulation group per row tile (a bank's
    # accumulate state is shared; two independent start/stop groups in one
    # bank corrupt each other).
    xpt = nc.dram_tensor("xpt", [128, 4, 2, 2, 512], f8, kind="ExternalInput").ap()
    xat = nc.dram_tensor("xat", [128, 4, 2, 2, 512], f8, kind="ExternalInput").ap()
    wna = nc.dram_tensor("wna", [128, 2, 2, 512], f8, kind="ExternalInput").ap()
    wtx = nc.dram_tensor("wtx", [128, 2, 2, 496], f8, kind="ExternalInput").ap()
    # Duplicate of row tile 0's stationary block: a tiny first DMA so the
    # first real matmul is not gated on the full xpt chunk transfer.
    xp00 = nc.dram_tensor("xp00", [128, 2, 2, 128], f8, kind="ExternalInput").ap()
    # Accumulators merged into one output: [se_a 32 | se_g 32 | rs_a 16 | rs_g 16]
    o_acc = nc.dram_tensor("o_acc", [128, 96], f32, kind="ExternalOutput").ap()
    o_cs = nc.dram_tensor("o_cs", [1, 2 * ASH], f32, kind="ExternalOutput").ap()

    with tile.TileContext(nc) as tc, ExitStack() as ctx:
        consts = ctx.enter_context(tc.tile_pool(name="consts", bufs=1))
        psums = ctx.enter_context(tc.tile_pool(name="psums", bufs=3, space="PSUM"))
        psum_cs = ctx.enter_context(
            tc.tile_pool(name="psum_cs", bufs=1, space="PSUM")
        )
        psum_h = ctx.enter_context(
            tc.tile_pool(name="psum_h", bufs=1, space="PSUM")
        )
        u16c = ctx.enter_context(tc.tile_pool(name="u16c", bufs=3))
        dpool = ctx.enter_context(tc.tile_pool(name="dpool", bufs=3))

        # Constants initialized on the GpSimd engine.
        warm = consts.tile([128, 512], bf16)
        nc.gpsimd.memset(warm, 0.0)
        ones = consts.tile([128, 1], bf16)
        nc.gpsimd.memset(ones, 1.0)

        # SBUF input tiles; DMA issue order == consumption order, smallest
        # first so the first real matmul starts as early as possible. Each
        # dma_start costs ~0.9us of SP sequencer issue time, so the count is
        # kept low (big tensors in halves).
        s_wna = consts.tile([128, 2, 2, 512], f8)
        s_xp00 = consts.tile([128, 2, 2, 128], f8)
        s_wtx = consts.tile([128, 2, 2, 496], f8)
        # wtx+wna ride the otherwise-idle ACT HWDGE queue (the SP issue
        # chain otherwise gates the first real matmul); the rest go on SP.
        nc.scalar.dma_start(out=s_wtx, in_=wtx)
        nc.scalar.dma_start(out=s_wna, in_=wna)
        nc.sync.dma_start(out=s_xp00, in_=xp00)
        s_xpt = consts.tile([128, 4, 2, 2, 512], f8)
        nc.sync.dma_start(out=s_xpt[:, 0:2], in_=xpt[:, 0:2])
        nc.sync.dma_start(out=s_xpt[:, 2:4], in_=xpt[:, 2:4])
        s_xat = consts.tile([128, 4, 2, 2, 512], f8)
        nc.sync.dma_start(out=s_xat[:, 0:2], in_=xat[:, 0:2])
        nc.sync.dma_start(out=s_xat[:, 2:4], in_=xat[:, 2:4])

        # Row-sum accumulators: one tile, disjoint column ranges per engine,
        # shipped with a single DMA at the end.
        acc = consts.tile([128, 96], f32)
        acc_se_a = acc[:, 0:32]
        acc_se_g = acc[:, 32:64]
        acc_rs_a = acc[:, 64:80]
        acc_rs_g = acc[:, 80:96]
        cs_sb = consts.tile([1, 2 * ASH], f32)

        # cse accumulates over 8 paired ones-matmuls in one PSUM bank;
        # the two 256-wide halves (even/odd row tiles) are summed on host.
        cse = psum_cs.tile([1, 2 * ASH], f32)
        # Dummy activation so the 1.3us exp table load happens at t~6us
        # instead of blocking the first real exp.
        twarm = consts.tile([128, 1], f32)
        nc.scalar.activation(twarm, ones, EXP)
        # ACT class-exp output is discarded; writing bf16 to SBUF avoids the
        # in-place PSUM read-modify-write penalty. One shared scratch is
        # enough (ACT program order serializes its writers).
        act_scr = consts.tile([128, CSH], bf16)
        # Heater target: dedicated spare bank, never read.
        heat = psum_h.tile([128, 512], f32)

        # PE warm-up: ramp the HAM clock while the first inputs stream in.
        for _ in range(N_WARM):
            nc.tensor.matmul(heat, warm[:, 0:128], warm, start=True, stop=True)
        heat_n = heat[:, 0:HEAT_N]
        warm_n = warm[:, 0:HEAT_N]

        dpairs = []
        for rt in range(32):
            src = s_xpt if rt < 16 else s_xat
            q, m0 = (rt % 16) // 4, ((rt % 16) % 4) * 128
            ps = psums.tile([128, PSW], f32, tag="ps")
            wid = PSW if rt < 16 else CSH
            for c in range(2):
                if rt == 0:
                    lhsT = s_xp00[:, c]
                else:
                    lhsT = src[:, q, c, :, m0 : m0 + 128]
                nc.tensor.matmul(
                    ps[:, 0:512],
                    lhsT,
                    s_wna[:, c],
                    start=(c == 0),
                    stop=(c == 1),
                    perf_mode=DR,
                )
                nc.tensor.matmul(
                    ps[:, 512:wid],
                    lhsT,
                    s_wtx[:, c, :, 0 : wid - 512],
                    start=(c == 0),
                    stop=(c == 1),
                    perf_mode=DR,
                )
            for _ in range(N_HEAT):
                nc.tensor.matmul(
                    heat_n, warm[:, 0:128], warm_n, start=True, stop=True
                )
            # Column sums of exp(alpha*D): one ones-matmul per PAIR of row
            # tiles, lagged two tiles so the PE never waits on the exps.
            if rt % 2 == 1 and 3 <= rt <= 17:
                pr = (rt - 3) // 2
                nc.tensor.matmul(
                    cse,
                    ones,
                    dpairs[pr].bitcast(bf16),
                    start=(pr == 0),
                    stop=(pr == 7),
                )

            # sum_j exp(SCALE * cos) for this row tile. ACT path: true exp,
            # written back in place to PSUM (output unused), fused accum.
            # DVE path: affine f32->uint16 (Schraudolph bits) to SBUF; the
            # GpSimd sums the bitcast bf16 values (it cannot touch PSUM,
            # but SBUF is fine and it is otherwise idle).
            if ASSIGN_CLS[rt] == "A":
                nc.scalar.activation(
                    act_scr,
                    ps[:, 0:CSH],
                    EXP,
                    scale=SCALE,
                    accum_out=acc_se_a[:, rt : rt + 1],
                )
            else:
                ebits = u16c.tile([128, CSH], u16, tag="u16c")
                nc.vector.tensor_scalar(
                    ebits, ps[:, 0:CSH], sch_a_cls, sch_b, MULT, ADD
                )
                eview = ebits.bitcast(bf16)
                nc.vector.tensor_scalar(
                    eview,
                    eview,
                    1.0,
                    None,
                    MULT,
                    ADD,
                    accum_out=acc_se_g[:, rt : rt + 1],
                )

            # exp(alpha * D): row sums via accum; the bf16 values land in
            # the current pair tile for the lagged column-sum matmul.
            if rt < 16:
                if rt % 2 == 0:
                    dpairs.append(
                        dpool.tile(
                            [128, 2, ASH], u16, tag="dpair", name=f"dp{rt}"
                        )
                    )
                dtgt = dpairs[-1][:, rt % 2, :]
                if ASSIGN_D[rt] == "A":
                    nc.scalar.activation(
                        dtgt.bitcast(bf16),
                        ps[:, CSH:PSW],
                        EXP,
                        scale=alpha,
                        accum_out=acc_rs_a[:, rt : rt + 1],
                    )
                else:
                    nc.vector.tensor_scalar(
                        dtgt, ps[:, CSH:PSW], sch_a_d, sch_b, MULT, ADD
                    )
                    dmm = dtgt.bitcast(bf16)
                    nc.vector.tensor_scalar(
                        dmm,
                        dmm,
                        1.0,
                        None,
                        MULT,
                        ADD,
                        accum_out=acc_rs_g[:, rt : rt + 1],
                    )

        nc.vector.tensor_copy(cs_sb, cse)
        nc.sync.dma_start(out=o_cs, in_=cs_sb)
        nc.sync.dma_start(out=o_acc, in_=acc)

    nc.compile()
    return nc


def _numpy_fallback(x, weight, w, b, w2, w3, b2, b3, label):
    """Exact float64 implementation of the reference (general w2/b2 path)."""
    x = np.asarray(x, np.float64)
    weight = np.asarray(weight, np.float64)
    label = np.asarray(label).astype(np.int64)
    w, b, w2, w3, b2, b3 = (float(v) for v in (w, b, w2, w3, b2, b3))

    def l2n(v):
        return v / np.maximum(np.linalg.norm(v, axis=-1, keepdims=True), 1e-12)

    def ce(logits, labels):
        m = logits.max(-1, keepdims=True)
        lse = np.log(np.exp(logits - m).sum(-1)) + m[:, 0]
        tgt = logits[np.arange(len(labels)), labels]
        return np.mean(lse - tgt)

    bsz = x.shape[0]
    xf = x.reshape(-1, NOUT)
    lab2 = np.repeat(label, 2)
    xn = l2n(xf)
    wn = l2n(weight)
    cosine = xn @ wn.T
    sine = np.sqrt(np.clip(1.0 - cosine * cosine, 0.0, 1.0))
    phi = cosine * _COS_M - sine * _SIN_M
    phi = np.where(cosine - _TH > 0, phi, cosine - _MM)
    one_hot = np.zeros_like(cosine)
    one_hot[np.arange(2 * bsz), lab2] = 1.0
    output = (one_hot * phi + (1.0 - one_hot) * cosine) * SCALE
    nlossS = ce(output, lab2)
    prec1 = np.mean(output.argmax(-1) == lab2) * 100.0

    cosr = cosine.reshape(bsz, 2, NCLS)

    def snorm(xr0, xr1, cos0, cos1):
        # xr0/cos0 = positive slot, xr1/cos1 = anchor slot
        out_dot = l2n(xr0) @ l2n(xr1).T
        COHORT = 101

        def stats(c):
            top = -np.partition(-c, COHORT - 1, axis=-1)[:, :COHORT]
            return top.mean(-1), top.std(-1, ddof=1)

        mean1, std1 = stats(cos1)
        mean2, std2 = stats(cos0)
        od1 = (out_dot - _hsig(mean1 * w2 + w3)[None, :]) / _hsig(
            std1 * b2 + b3
        )[None, :]
        od2 = (out_dot - _hsig(mean2 * w2 + w3)[:, None]) / _hsig(
            std2 * b2 + b3
        )[:, None]
        csm = 0.5 * (od1 + od2) * w + b
        return ce(csm, np.arange(bsz))

    xr = xf.reshape(bsz, 2, NOUT)
    p1 = snorm(xr[:, 0], xr[:, 1], cosr[:, 0], cosr[:, 1])
    p2 = snorm(xr[:, 1], xr[:, 0], cosr[:, 1], cosr[:, 0])
    nlossP = 0.5 * (p1 + p2)
    return np.asarray([nlossS + nlossP, prec1], np.float32)


def _pack_dr(opT):
    """[512, N] fp8 operand -> [128, N/512, 2, 2, 512] DoubleRow DMA layout."""
    n = opT.shape[1]
    # [c, r, p, col] with k = c*256 + r*128 + p
    a = opT.reshape(2, 2, 128, n)
    # -> [p, q, c, r, m]
    a = a.transpose(2, 0, 1, 3).reshape(128, 2, 2, n // 512, 512)
    return np.ascontiguousarray(a.transpose(0, 3, 1, 2, 4))


def kernel(x, weight, w, b, w2, w3, b2, b3, label):
    global last_results
    w_f, b_f, w2_f, w3_f, b2_f, b3_f = (
        float(np.asarray(v)) for v in (w, b, w2, w3, b2, b3)
    )
    if w2_f != 0.0 or b2_f != 0.0 or _hsig(b3_f) <= 0.0:
        return _numpy_fallback(x, weight, w, b, w2, w3, b2, b3, label)

    from concourse.bass_utils import run_bass_kernel_spmd

    x = np.asarray(x, np.float32)
    weight = np.asarray(weight, np.float32)
    label = np.asarray(label).astype(np.int64)
    alpha = w_f / _hsig(b3_f)

    # ---- host prep: normalize, quantize to fp8, pack DMA layouts ----
    xf = x.reshape(R, NOUT)
    xn = xf / np.maximum(np.linalg.norm(xf, axis=-1, keepdims=True), 1e-12)
    wn = weight / np.maximum(np.linalg.norm(weight, axis=-1, keepdims=True), 1e-12)
    xn8 = xn.astype(ml_dtypes.float8_e4m3)
    wn8 = wn.astype(ml_dtypes.float8_e4m3)

    XpT = np.ascontiguousarray(xn8[0::2].T)  # [512, 2048]
    XaT = np.ascontiguousarray(xn8[1::2].T)  # [512, 2048]
    WnT = np.zeros((NOUT, NCORES * CSH), ml_dtypes.float8_e4m3)
    WnT[:, :NCLS] = wn8.T

    xpt_p = _pack_dr(XpT)
    xat_p = _pack_dr(XaT)
    xp00_p = np.ascontiguousarray(xpt_p[:, 0, :, :, 0:128])
    in_maps = []
    for k in range(NCORES):
        wsh = WnT[:, k * CSH : (k + 1) * CSH]
        # [512, 496] fused tail: class cols 512:752 then the anchor shard
        tx = np.concatenate(
            [wsh[:, 512:CSH], XaT[:, k * ASH : (k + 1) * ASH]], axis=1
        )
        in_maps.append(
            {
                "xpt": xpt_p,
                "xat": xat_p,
                "xp00": xp00_p,
                # [p][c][r][col] packing for the 512/496-wide operands
                "wna": np.ascontiguousarray(
                    wsh[:, 0:512].reshape(2, 2, 128, 512).transpose(2, 0, 1, 3)
                ),
                "wtx": np.ascontiguousarray(
                    tx.reshape(2, 2, 128, 496).transpose(2, 0, 1, 3)
                ),
            }
        )

    key = ("prog", alpha)
    if key not in _cache:
        _cache[key] = _build_program(alpha)
    nc = _cache[key]

    res = run_bass_kernel_spmd(nc, in_maps, list(range(NCORES)))
    last_results = res

    # ---- host combine (float64) ----
    # Row tiling: rt < 16 -> positive rows (xf rows 0,2,...), rt >= 16 ->
    # anchor rows; row = (rt % 16) * 128 + p.
    pad_a = 1.0
    pad_d = float(_sch_exp_np(np.zeros(1), SCALE)[0])
    se = np.zeros((128, 32), np.float64)
    rowSE = np.zeros((B,), np.float64)
    cse = np.zeros((B,), np.float64)
    for k in range(NCORES):
        r = res.results[k]
        oacc = np.asarray(r["o_acc"], np.float64)
        se_a, se_g = oacc[:, 0:32], oacc[:, 32:64]
        rs_a, rs_g = oacc[:, 64:80], oacc[:, 80:96]
        for rt in range(32):
            col = se_a[:, rt] if ASSIGN_CLS[rt] == "A" else se_g[:, rt]
            if k == NCORES - 1:
                col = col - NPAD * (pad_a if ASSIGN_CLS[rt] == "A" else pad_d)
            se[:, rt] += col
        for rt in range(16):
            rowSE[rt * 128 : (rt + 1) * 128] += (
                rs_a[:, rt] if ASSIGN_D[rt] == "A" else rs_g[:, rt]
            )
        ocs = np.asarray(r["o_cs"], np.float64)[0]
        cse[k * ASH : (k + 1) * ASH] = ocs[:ASH] + ocs[ASH:]

    def tiles_to_rows(t):  # t: [128, 32] -> [4096] in xf row order
        pos = t[:, :16].T.reshape(-1)
        anc = t[:, 16:].T.reshape(-1)
        out = np.empty(R, np.float64)
        out[0::2] = pos
        out[1::2] = anc
        return out

    sumexp = tiles_to_rows(se)

    # Target cosines / diag from the same fp8-quantized operands.
    xn8f = xn8.astype(np.float64)
    wn8f = wn8.astype(np.float64)
    lab2 = np.repeat(label, 2)
    c_t = np.einsum("ij,ij->i", xn8f, wn8f[lab2])
    d = np.einsum("ij,ij->i", xn8f[0::2], xn8f[1::2])

    # Device-replicated target term (engine of the row's tile).
    eng = np.empty(R, dtype="U1")
    for rt in range(32):
        base = 0 if rt < 16 else 1
        rows = 2 * ((rt % 16) * 128 + np.arange(128)) + base
        eng[rows] = ASSIGN_CLS[rt]
    t_dev = np.where(
        eng == "A", np.exp(SCALE * c_t), _sch_exp_np(c_t, SCALE)
    )

    sine = np.sqrt(np.clip(1.0 - c_t * c_t, 0.0, 1.0))
    phi = np.where(c_t - _TH > 0, c_t * _COS_M - sine * _SIN_M, c_t - _MM)
    se_no_t = np.maximum(sumexp - t_dev, 1e-300)
    lse = np.log(se_no_t + np.exp(SCALE * phi))
    nlossS = np.mean(lse - SCALE * phi)

    # prec1: argmax==label iff phi > max_{j!=label} cos_j. From the exp-sum,
    # max_{j!=label} >= log(se_no_t/5993)/SCALE; rows above that bound
    # (minus a safety margin for the ~3% Schraudolph error) get an exact check.
    lb = (np.log(se_no_t) - math.log(NCLS - 1)) / SCALE
    amb = phi > lb - 0.004
    prec_bits = np.zeros(R, bool)
    if amb.any():
        idx = np.where(amb)[0]
        cosf = xn8f[idx] @ wn8f.T
        cosf[np.arange(len(idx)), lab2[idx]] = -np.inf
        prec_bits[idx] = phi[idx] > cosf.max(axis=1)
    prec1 = 100.0 * prec_bits.mean()

    p1 = np.mean(np.log(rowSE) - alpha * d)
    p2 = np.mean(np.log(cse) - alpha * d)
    nlossP = 0.5 * (p1 + p2)

    return np.asarray([nlossS + nlossP, prec1], np.float32)


# revision 62
# speedup vs baseline: 1.0592x; 1.0592x over previous
"""Trainium2 Bass kernel for nn_LossFunction_2740189135094 (AAM-softmax +
score-normalized angle-proto speaker loss).

Contract: kernel(**inputs) takes FULL unsharded inputs (as produced by the
reference setup_inputs) and returns the full output: a (2,) float32 array
[nlossS + nlossP, prec1].

Strategy (8 NeuronCores, class-sharded; tiny partial outputs merged on host):
  Each core owns 752 of the (padded-to-6016) 5994 classes and 256 of the
  2048 anchors. Per 128-row tile the PE computes, in one fused [128, 1008]
  PSUM tile, cosine vs the class shard (fp8 DoubleRow) and - for positive
  rows - the angle-proto similarity D vs the anchor shard. The row-wise
  sum(exp(30*cos)) is computed by BOTH the ACT engine (true exp + fused
  accumulate) and the DVE (Schraudolph bit-trick exp: affine f32->uint16,
  bitcast to bf16, 4x-mode accumulate), split per a static assignment so the
  two engines finish together. exp(alpha*D) row sums come from the same
  machinery; anchor-column sums come from a ones-vector matmul on the PE
  accumulated across row tiles in PSUM.

  There is no on-device row max: prec1 = mean(phi > max_{j!=label} cos_j)
  is decided on the host from the row-wise sum(exp): max >= log(sum/5993)/30.
  For this loss phi sits far below that bound; rows too close to the bound
  (never, in practice) fall back to an exact host check.

  Host does: l2-normalization, fp8-e4m3 quantization, layout packing (so
  every DMA is contiguous per partition - the input load is descriptor-bound
  otherwise), the label-gathered target cosines from the same fp8 operands,
  and the final logs/means in float64.

The top-k cohort statistics in the reference are multiplied by w2/b2; for
the actual inputs w2 == b2 == 0, so csm is an affine function of out_dot and
p2's matrix is exactly p1's transpose. If w2/b2 were nonzero we fall back to
an exact numpy implementation.
"""

import math
import sys

import numpy as np

for _p in ("/opt/trn_rl_repo", "/opt/pypackages"):
    if _p not in sys.path:
        sys.path.insert(0, _p)

import ml_dtypes  # noqa: E402

NOUT = 512
NCLS = 5994
B = 2048
R = 4096  # 2 * B rows
NCORES = 8
CSH = 752  # padded class shard: 8 * 752 = 6016 >= 5994
NPAD = NCORES * CSH - NCLS  # 22 zero-padded classes on the last core
ASH = B // NCORES  # 256 anchors per core
MARGIN = 0.2
SCALE = 30.0
PSW = CSH + ASH  # fused psum tile width: classes + anchor shard

_COS_M = math.cos(MARGIN)
_SIN_M = math.sin(MARGIN)
_TH = math.cos(math.pi - MARGIN)
_MM = math.sin(math.pi - MARGIN) * MARGIN

LOG2E = 1.4426950408889634
# Schraudolph bf16 exp: exp(s*c) ~= bitcast_bf16(uint16(c*(128*s*log2e) + BC)).
# BC is bias-free for sums: 128*(127 - log2(E_f[(1+f)/2^f])).
SCH_B = 128.0 * 127.0 - 128.0 * math.log2(1.0407419227)

# Engine assignment: 'A' = ACT (true exp, fused accum), 'D' = DVE
# (Schraudolph affine+convert + bf16 accumulate; the Pool engine supports
# neither PSUM access nor TensorScalar, so only these two engines can
# consume matmul results). Balanced from measured slice times: ACT class
# tile ~1069ns (exp 861 + accum read 208), DVE class pair ~1792ns (the
# accumulate lowers to TENSOR_SCALAR_CACHE_REDUCE, which runs 1 elem/cycle
# - no 2x/4x modes), ACT D tile ~630ns, DVE D pair ~670ns.
N_ACT_CLS = 22
ASSIGN_CLS = [
    "A" if (i * N_ACT_CLS) // 32 != ((i + 1) * N_ACT_CLS) // 32 else "D"
    for i in range(32)
]
# rt0 on ACT (it can start right after the early table load); rt30 on DVE
# (the formula puts three ACT tiles in a row at the tail, serializing it).
ASSIGN_CLS[0], ASSIGN_CLS[30] = "A", "D"
# The D-tile 'A' slots sit on ALL DVE-class rows so every PSUM tile's two
# consumers run on different engines concurrently (a tile whose class pair
# AND D pair both land on DVE holds the 3-deep PSUM pool ~2.4us and stalls
# the PE).
_dve_cls_rts = [i for i in range(16) if ASSIGN_CLS[i] == "D"]
ASSIGN_D = ["A" if i in set(_dve_cls_rts) else "D" for i in range(16)]
# A matmul instruction costs ~N_out cycles regardless of contraction depth
# (DoubleRow packs k=256 per instruction; it does NOT halve the per-
# instruction stream time), so the PE floor here is ~25us and the PE is the
# natural pacer. One small heater matmul per row tile keeps the HAM clock
# governor at 2.4 GHz (without them the clock was observed stuck at
# 0.65-1.2 GHz for the whole run, doubling every matmul).
N_HEAT = 1
HEAT_N = 128
N_WARM = 4

_cache: dict = {}

# Results of the last device run (for the test harness to inspect timing).
last_results = None


def _hsig(v):
    return np.clip((v + 3.0) / 6.0, 0.0, 1.0)


def _sch_exp_np(c, s):
    """Replicate the device's Schraudolph exp in numpy (float64 out)."""
    a = np.float32(128.0 * s * LOG2E)
    t = np.asarray(c, np.float32) * a + np.float32(SCH_B)
    i = np.round(t).astype(np.uint16)
    return i.view(ml_dtypes.bfloat16).astype(np.float64)


def _build_program(alpha):
    import concourse.mybir as mybir
    import concourse.tile as tile
    from concourse import bacc
    from contextlib import ExitStack

    bf16 = mybir.dt.bfloat16
    u16 = mybir.dt.uint16
    f8 = mybir.dt.float8e4
    f32 = mybir.dt.float32
    DR = mybir.MatmulPerfMode.DoubleRow
    EXP = mybir.ActivationFunctionType.Exp
    MULT = mybir.AluOpType.mult
    ADD = mybir.AluOpType.add
    MAX = mybir.AluOpType.max

    sch_a_cls = float(np.float32(128.0 * SCALE * LOG2E))
    sch_a_d = float(np.float32(128.0 * alpha * LOG2E))
    sch_b = float(np.float32(SCH_B))

    nc = bacc.Bacc(
        "TRN2", target_bir_lowering=False, debug=False, num_devices=NCORES
    )
    # Inputs are pre-packed on host so each DMA is one contiguous run per
    # partition (the load is descriptor-count-bound otherwise).
    # Layout [p][q][c][r][m]: element = operand[k, col] with k = c*256+r*128+p
    # (contraction index, DoubleRow-packed) and col = q*512+m.
    # wtx fuses the class tail (240 cols) with the anchor shard (256 cols) so
    # PSUM bank 1 holds exactly one accumulation group per row tile (a bank's
    # accumulate state is shared; two independent start/stop groups in one
    # bank corrupt each other).
    xpt = nc.dram_tensor("xpt", [128, 4, 2, 2, 512], f8, kind="ExternalInput").ap()
    xat = nc.dram_tensor("xat", [128, 4, 2, 2, 512], f8, kind="ExternalInput").ap()
    wna = nc.dram_tensor("wna", [128, 2, 2, 512], f8, kind="ExternalInput").ap()
    wtx = nc.dram_tensor("wtx", [128, 2, 2, 496], f8, kind="ExternalInput").ap()
    # Duplicate of row tile 0's stationary block: a tiny first DMA so the
    # first real matmul is not gated on the full xpt chunk transfer.
    xp00 = nc.dram_tensor("xp00", [128, 2, 2, 128], f8, kind="ExternalInput").ap()
    # Accumulators merged into one output: [se_a 32 | se_g 32 | rs_a 16 | rs_g 16]
    o_acc = nc.dram_tensor("o_acc", [128, 96], f32, kind="ExternalOutput").ap()
    o_cs = nc.dram_tensor("o_cs", [1, 2 * ASH], f32, kind="ExternalOutput").ap()

    with tile.TileContext(nc) as tc, ExitStack() as ctx:
        consts = ctx.enter_context(tc.tile_pool(name="consts", bufs=1))
        psums = ctx.enter_context(tc.tile_pool(name="psums", bufs=3, space="PSUM"))
        psum_cs = ctx.enter_context(
            tc.tile_pool(name="psum_cs", bufs=1, space="PSUM")
        )
        psum_h = ctx.enter_context(
            tc.tile_pool(name="psum_h", bufs=1, space="PSUM")
        )
        u16c = ctx.enter_context(tc.tile_pool(name="u16c", bufs=3))
        dpool = ctx.enter_context(tc.tile_pool(name="dpool", bufs=3))

        # Constants initialized on the GpSimd engine.
        warm = consts.tile([128, 512], bf16)
        nc.gpsimd.memset(warm, 0.0)
        ones = consts.tile([128, 1], bf16)
        nc.gpsimd.memset(ones, 1.0)

        # SBUF input tiles; DMA issue order == consumption order, smallest
        # first so the first real matmul starts as early as possible. Each
        # dma_start costs ~0.9us of SP sequencer issue time, so the count is
        # kept low (big tensors in halves).
        s_wna = consts.tile([128, 2, 2, 512], f8)
        s_xp00 = consts.tile([128, 2, 2, 128], f8)
        s_wtx = consts.tile([128, 2, 2, 496], f8)
        # wtx rides the otherwise-idle ACT HWDGE queue; the rest go on SP.
        nc.scalar.dma_start(out=s_wtx, in_=wtx)
        nc.sync.dma_start(out=s_wna, in_=wna)
        nc.sync.dma_start(out=s_xp00, in_=xp00)
        s_xpt = consts.tile([128, 4, 2, 2, 512], f8)
        nc.sync.dma_start(out=s_xpt[:, 0:2], in_=xpt[:, 0:2])
        nc.sync.dma_start(out=s_xpt[:, 2:4], in_=xpt[:, 2:4])
        s_xat = consts.tile([128, 4, 2, 2, 512], f8)
        nc.sync.dma_start(out=s_xat[:, 0:2], in_=xat[:, 0:2])
        nc.sync.dma_start(out=s_xat[:, 2:4], in_=xat[:, 2:4])

        # Row-sum accumulators: one tile, disjoint column ranges per engine,
        # shipped with a single DMA at the end.
        acc = consts.tile([128, 96], f32)
        acc_se_a = acc[:, 0:32]
        acc_se_g = acc[:, 32:64]
        acc_rs_a = acc[:, 64:80]
        acc_rs_g = acc[:, 80:96]
        cs_sb = consts.tile([1, 2 * ASH], f32)

        # cse accumulates over 8 paired ones-matmuls in one PSUM bank;
        # the two 256-wide halves (even/odd row tiles) are summed on host.
        cse = psum_cs.tile([1, 2 * ASH], f32)
        # Dummy activation so the 1.3us exp table load happens at t~6us
        # instead of blocking the first real exp.
        twarm = consts.tile([128, 1], f32)
        nc.scalar.activation(twarm, ones, EXP)
        # ACT class-exp output is discarded; writing bf16 to SBUF avoids the
        # in-place PSUM read-modify-write penalty. One shared scratch is
        # enough (ACT program order serializes its writers).
        act_scr = consts.tile([128, CSH], bf16)
        # Heater target: dedicated spare bank, never read.
        heat = psum_h.tile([128, 512], f32)

        # PE warm-up: ramp the HAM clock while the first inputs stream in.
        for _ in range(N_WARM):
            nc.tensor.matmul(heat, warm[:, 0:128], warm, start=True, stop=True)
        heat_n = heat[:, 0:HEAT_N]
        warm_n = warm[:, 0:HEAT_N]

        dpairs = []
        for rt in range(32):
            src = s_xpt if rt < 16 else s_xat
            q, m0 = (rt % 16) // 4, ((rt % 16) % 4) * 128
            ps = psums.tile([128, PSW], f32, tag="ps")
            wid = PSW if rt < 16 else CSH
            for c in range(2):
                if rt == 0:
                    lhsT = s_xp00[:, c]
                else:
                    lhsT = src[:, q, c, :, m0 : m0 + 128]
                nc.tensor.matmul(
                    ps[:, 0:512],
                    lhsT,
                    s_wna[:, c],
                    start=(c == 0),
                    stop=(c == 1),
                    perf_mode=DR,
                )
                nc.tensor.matmul(
                    ps[:, 512:wid],
                    lhsT,
                    s_wtx[:, c, :, 0 : wid - 512],
                    start=(c == 0),
                    stop=(c == 1),
                    perf_mode=DR,
                )
            for _ in range(N_HEAT):
                nc.tensor.matmul(
                    heat_n, warm[:, 0:128], warm_n, start=True, stop=True
                )
            # Column sums of exp(alpha*D): one ones-matmul per PAIR of row
            # tiles, lagged two tiles so the PE never waits on the exps.
            if rt % 2 == 1 and 3 <= rt <= 17:
                pr = (rt - 3) // 2
                nc.tensor.matmul(
                    cse,
                    ones,
                    dpairs[pr].bitcast(bf16),
                    start=(pr == 0),
                    stop=(pr == 7),
                )

            # sum_j exp(SCALE * cos) for this row tile. ACT path: true exp,
            # written back in place to PSUM (output unused), fused accum.
            # DVE path: affine f32->uint16 (Schraudolph bits) to SBUF; the
            # GpSimd sums the bitcast bf16 values (it cannot touch PSUM,
            # but SBUF is fine and it is otherwise idle).
            if ASSIGN_CLS[rt] == "A":
                nc.scalar.activation(
                    act_scr,
                    ps[:, 0:CSH],
                    EXP,
                    scale=SCALE,
                    accum_out=acc_se_a[:, rt : rt + 1],
                )
            else:
                ebits = u16c.tile([128, CSH], u16, tag="u16c")
                nc.vector.tensor_scalar(
                    ebits, ps[:, 0:CSH], sch_a_cls, sch_b, MULT, ADD
                )
                eview = ebits.bitcast(bf16)
                nc.vector.tensor_scalar(
                    eview,
                    eview,
                    1.0,
                    None,
                    MULT,
                    ADD,
                    accum_out=acc_se_g[:, rt : rt + 1],
                )

            # exp(alpha * D): row sums via accum; the bf16 values land in
            # the current pair tile for the lagged column-sum matmul.
            if rt < 16:
                if rt % 2 == 0:
                    dpairs.append(
                        dpool.tile(
                            [128, 2, ASH], u16, tag="dpair", name=f"dp{rt}"
                        )
                    )
                dtgt = dpairs[-1][:, rt % 2, :]
                if ASSIGN_D[rt] == "A":
                    nc.scalar.activation(
                        dtgt.bitcast(bf16),
                        ps[:, CSH:PSW],
                        EXP,
                        scale=alpha,
                        accum_out=acc_rs_a[:, rt : rt + 1],
                    )
                else:
                    nc.vector.tensor_scalar(
                        dtgt, ps[:, CSH:PSW], sch_a_d, sch_b, MULT, ADD
                    )
                    dmm = dtgt.bitcast(bf16)
                    nc.vector.tensor_scalar(
                        dmm,
                        dmm,
                        1.0,
                        None,
                        MULT,
                        ADD,
                        accum_out=acc_rs_g[:, rt : rt + 1],
                    )

        nc.vector.tensor_copy(cs_sb, cse)
        nc.sync.dma_start(out=o_cs, in_=cs_sb)
        nc.sync.dma_start(out=o_acc, in_=acc)

    nc.compile()
    return nc


def _numpy_fallback(x, weight, w, b, w2, w3, b2, b3, label):
    """Exact float64 implementation of the reference (general w2/b2 path)."""
    x = np.asarray(x, np.float64)
    weight = np.asarray(weight, np.float64)
    label = np.asarray(label).astype(np.int64)
    w, b, w2, w3, b2, b3 = (float(v) for v in (w, b, w2, w3, b2, b3))

    def l2n(v):
        return v / np.maximum(np.linalg.norm(v, axis=-1, keepdims=True), 1e-12)

    def ce(logits, labels):
        m = logits.max(-1, keepdims=True)
        lse = np.log(np.exp(logits - m).sum(-1)) + m[:, 0]
        tgt = logits[np.arange(len(labels)), labels]
        return np.mean(lse - tgt)

    bsz = x.shape[0]
    xf = x.reshape(-1, NOUT)
    lab2 = np.repeat(label, 2)
    xn = l2n(xf)
    wn = l2n(weight)
    cosine = xn @ wn.T
    sine = np.sqrt(np.clip(1.0 - cosine * cosine, 0.0, 1.0))
    phi = cosine * _COS_M - sine * _SIN_M
    phi = np.where(cosine - _TH > 0, phi, cosine - _MM)
    one_hot = np.zeros_like(cosine)
    one_hot[np.arange(2 * bsz), lab2] = 1.0
    output = (one_hot * phi + (1.0 - one_hot) * cosine) * SCALE
    nlossS = ce(output, lab2)
    prec1 = np.mean(output.argmax(-1) == lab2) * 100.0

    cosr = cosine.reshape(bsz, 2, NCLS)

    def snorm(xr0, xr1, cos0, cos1):
        # xr0/cos0 = positive slot, xr1/cos1 = anchor slot
        out_dot = l2n(xr0) @ l2n(xr1).T
        COHORT = 101

        def stats(c):
            top = -np.partition(-c, COHORT - 1, axis=-1)[:, :COHORT]
            return top.mean(-1), top.std(-1, ddof=1)

        mean1, std1 = stats(cos1)
        mean2, std2 = stats(cos0)
        od1 = (out_dot - _hsig(mean1 * w2 + w3)[None, :]) / _hsig(
            std1 * b2 + b3
        )[None, :]
        od2 = (out_dot - _hsig(mean2 * w2 + w3)[:, None]) / _hsig(
            std2 * b2 + b3
        )[:, None]
        csm = 0.5 * (od1 + od2) * w + b
        return ce(csm, np.arange(bsz))

    xr = xf.reshape(bsz, 2, NOUT)
    p1 = snorm(xr[:, 0], xr[:, 1], cosr[:, 0], cosr[:, 1])
    p2 = snorm(xr[:, 1], xr[:, 0], cosr[:, 1], cosr[:, 0])
    nlossP = 0.5 * (p1 + p2)
    return np.asarray([nlossS + nlossP, prec1], np.float32)


def _pack_dr(opT):
    """[512, N] fp8 operand -> [128, N/512, 2, 2, 512] DoubleRow DMA layout."""
    n = opT.shape[1]
    # [c, r, p, col] with k = c*256 + r*128 + p
    a = opT.reshape(2, 2, 128, n)
    # -> [p, q, c, r, m]
    a = a.transpose(2, 0, 1, 3).reshape(128, 2, 2, n // 512, 512)
    return np.ascontiguousarray(a.transpose(0, 3, 1, 2, 4))


def kernel(x, weight, w, b, w2, w3, b2, b3, label):
    global last_results
    w_f, b_f, w2_f, w3_f, b2_f, b3_f = (
        float(np.asarray(v)) for v in (w, b, w2, w3, b2, b3)
    )
    if w2_f != 0.0 or b2_f != 0.0 or _hsig(b3_f) <= 0.0:
        return _numpy_fallback(x, weight, w, b, w2, w3, b2, b3, label)

    from concourse.bass_utils import run_bass_kernel_spmd

    x = np.asarray(x, np.float32)
    weight = np.asarray(weight, np.float32)
    label = np.asarray(label).astype(np.int64)
    alpha = w_f / _hsig(b3_f)

    # ---- host prep: normalize, quantize to fp8, pack DMA layouts ----
    xf = x.reshape(R, NOUT)
    xn = xf / np.maximum(np.linalg.norm(xf, axis=-1, keepdims=True), 1e-12)
    wn = weight / np.maximum(np.linalg.norm(weight, axis=-1, keepdims=True), 1e-12)
    xn8 = xn.astype(ml_dtypes.float8_e4m3)
    wn8 = wn.astype(ml_dtypes.float8_e4m3)

    XpT = np.ascontiguousarray(xn8[0::2].T)  # [512, 2048]
    XaT = np.ascontiguousarray(xn8[1::2].T)  # [512, 2048]
    WnT = np.zeros((NOUT, NCORES * CSH), ml_dtypes.float8_e4m3)
    WnT[:, :NCLS] = wn8.T

    xpt_p = _pack_dr(XpT)
    xat_p = _pack_dr(XaT)
    xp00_p = np.ascontiguousarray(xpt_p[:, 0, :, :, 0:128])
    in_maps = []
    for k in range(NCORES):
        wsh = WnT[:, k * CSH : (k + 1) * CSH]
        # [512, 496] fused tail: class cols 512:752 then the anchor shard
        tx = np.concatenate(
            [wsh[:, 512:CSH], XaT[:, k * ASH : (k + 1) * ASH]], axis=1
        )
        in_maps.append(
            {
                "xpt": xpt_p,
                "xat": xat_p,
                "xp00": xp00_p,
                # [p][c][r][col] packing for the 512/496-wide operands
                "wna": np.ascontiguousarray(
                    wsh[:, 0:512].reshape(2, 2, 128, 512).transpose(2, 0, 1, 3)
                ),
                "wtx": np.ascontiguousarray(
                    tx.reshape(2, 2, 128, 496).transpose(2, 0, 1, 3)
                ),
            }
        )

    key = ("prog", alpha)
    if key not in _cache:
        _cache[key] = _build_program(alpha)
    nc = _cache[key]

    res = run_bass_kernel_spmd(nc, in_maps, list(range(NCORES)))
    last_results = res

    # ---- host combine (float64) ----
    # Row tiling: rt < 16 -> positive rows (xf rows 0,2,...), rt >= 16 ->
    # anchor rows; row = (rt % 16) * 128 + p.
    pad_a = 1.0
    pad_d = float(_sch_exp_np(np.zeros(1), SCALE)[0])
    se = np.zeros((128, 32), np.float64)
    rowSE = np.zeros((B,), np.float64)
    cse = np.zeros((B,), np.float64)
    for k in range(NCORES):
        r = res.results[k]
        oacc = np.asarray(r["o_acc"], np.float64)
        se_a, se_g = oacc[:, 0:32], oacc[:, 32:64]
        rs_a, rs_g = oacc[:, 64:80], oacc[:, 80:96]
        for rt in range(32):
            col = se_a[:, rt] if ASSIGN_CLS[rt] == "A" else se_g[:, rt]
            if k == NCORES - 1:
                col = col - NPAD * (pad_a if ASSIGN_CLS[rt] == "A" else pad_d)
            se[:, rt] += col
        for rt in range(16):
            rowSE[rt * 128 : (rt + 1) * 128] += (
                rs_a[:, rt] if ASSIGN_D[rt] == "A" else rs_g[:, rt]
            )
        ocs = np.asarray(r["o_cs"], np.float64)[0]
        cse[k * ASH : (k + 1) * ASH] = ocs[:ASH] + ocs[ASH:]

    def tiles_to_rows(t):  # t: [128, 32] -> [4096] in xf row order
        pos = t[:, :16].T.reshape(-1)
        anc = t[:, 16:].T.reshape(-1)
        out = np.empty(R, np.float64)
        out[0::2] = pos
        out[1::2] = anc
        return out

    sumexp = tiles_to_rows(se)

    # Target cosines / diag from the same fp8-quantized operands.
    xn8f = xn8.astype(np.float64)
    wn8f = wn8.astype(np.float64)
    lab2 = np.repeat(label, 2)
    c_t = np.einsum("ij,ij->i", xn8f, wn8f[lab2])
    d = np.einsum("ij,ij->i", xn8f[0::2], xn8f[1::2])

    # Device-replicated target term (engine of the row's tile).
    eng = np.empty(R, dtype="U1")
    for rt in range(32):
        base = 0 if rt < 16 else 1
        rows = 2 * ((rt % 16) * 128 + np.arange(128)) + base
        eng[rows] = ASSIGN_CLS[rt]
    t_dev = np.where(
        eng == "A", np.exp(SCALE * c_t), _sch_exp_np(c_t, SCALE)
    )

    sine = np.sqrt(np.clip(1.0 - c_t * c_t, 0.0, 1.0))
    phi = np.where(c_t - _TH > 0, c_t * _COS_M - sine * _SIN_M, c_t - _MM)
    se_no_t = np.maximum(sumexp - t_dev, 1e-300)
    lse = np.log(se_no_t + np.exp(SCALE * phi))
    nlossS = np.mean(lse - SCALE * phi)

    # prec1: argmax==label iff phi > max_{j!=label} cos_j. From the exp-sum,
    # max_{j!=label} >= log(se_no_t/5993)/SCALE; rows above that bound
    # (minus a safety margin for the ~3% Schraudolph error) get an exact check.
    lb = (np.log(se_no_t) - math.log(NCLS - 1)) / SCALE
    amb = phi > lb - 0.004
    prec_bits = np.zeros(R, bool)
    if amb.any():
        idx = np.where(amb)[0]
        cosf = xn8f[idx] @ wn8f.T
        cosf[np.arange(len(idx)), lab2[idx]] = -np.inf
        prec_bits[idx] = phi[idx] > cosf.max(axis=1)
    prec1 = 100.0 * prec_bits.mean()

    p1 = np.mean(np.log(rowSE) - alpha * d)
    p2 = np.mean(np.log(cse) - alpha * d)
    nlossP = 0.5 * (p1 + p2)

    return np.asarray([nlossS + nlossP, prec1], np.float32)
